# revision 35
# baseline (speedup 1.0000x reference)
"""Multi-head attention (B=2, N=2048, C=1024, H=16) on 8 TRN2 NeuronCores.

Sharding: (batch, head-quad) — core c handles batch c//4 and heads
[4*(c%4), 4*(c%4)+4).  Each core loads only x[b] (bf16, 4.2MB), runs one
continuous attention phase of 8 waves (query-block, head-pair), then
8-core AllToAlls token-shard h for the output projection (each core
projects a 64-token slice of BOTH batches per query-block group).

All matmuls bf16 (fp8 was measured at ~+2% output max-err per quantized
stage — softmax averaging shrinks signal and noise equally, so fp8 noise
passes through ~1:1 and blows the 2e-2 budget).

Per-core dataflow (bf16 matmuls, fp32 softmax-normalizer chain):
  x^T (host-pretransposed) --matmul w/ W^T shards--> Q^T,K^T [128,N] x2 hp
  V in [tok, ch] layout, 4 heads strided into vau (65-col slots, ones col)
  S^T[k,q] per 128-key chunk: d=64 contraction, head-pair row-group pairs
  P^T = exp(0.125*S^T) (ScalarE, one [128,1024] ACTIVATE per chunk)
  h_aug^T[65,q] += [V|1].T @ P^T  per (chunk, head); row 64 = denominator
  normalize via DMA-reshaped reciprocal + partition_broadcast + mul
  per-(qb,hp) half-AllToAll [8,128,64] right at wave end
  proj: batched pl loads (stationary [128, 2*64]) @ pw + DVE bias add

Schedule notes:
  - Waves ordered (0,0),(1,0),(0,1),(1,1),(2,0),(2,1),(3,0),(3,1); QKV
    chains and projections run as PE fillers inside the exp-bound waves
    (rate 4 while QKV remains, rate 2 after).
  - Half-A2As per (qb,hp) halve the tail-critical collective and start
    the hp0 half a wave early.  A tiny priming AllToAll at startup
    absorbs the ~11us first-collective setup cost.
  - proj pl/out DMAs ride the Scalar engine's otherwise-empty HWDGE ring:
    their collective-completion descriptor waits must not head-of-line
    block the sync ring (normalize DMAs) or gpsimd (triggers/broadcasts).
  - proj matmuls are gated on a wave counter (collective in flight would
    stall the in-order PE queue); "pause" yields in the filler protocol
    implement the gating without consuming pull budget.
  - Dummy keep-warm matmuls before the drain hold the HAM clock gate at
    full rate across the final A2A wait.
  - First device execution after NEFF load has a cold-start race in the
    collective path; kernel() runs a warmup execution and discards it.
"""

import numpy as np
import ml_dtypes

import concourse.mybir as mybir
import concourse.tile as tile
from concourse import bacc
from concourse.bass_utils import run_bass_kernel_spmd

F32 = mybir.dt.float32
BF16 = mybir.dt.bfloat16
EXP = mybir.ActivationFunctionType.Exp

N_CORES = 8
B = 2
C = 1024
H = 16
D = 64
HPC = 4                      # heads per core
CH = HPC * D                 # channels per core (256)
KT_C = C // 128              # contraction chunks (8)


def build_program(N=2048, n_cores=N_CORES):
    assert N % 512 == 0
    QB = N // 512            # query blocks (4)
    NK = N // 128            # key chunks (16)
    scale = float(D) ** -0.5
    WAVES = [(0, 0), (1, 0), (0, 1), (1, 1), (2, 0), (2, 1), (3, 0), (3, 1)]
    TOKG = 512 // n_cores    # tokens per core per group (64)

    nc = bacc.Bacc("TRN2", target_bir_lowering=False, debug=False,
                   num_devices=n_cores)

    xT_d = nc.dram_tensor("xT", [C, N], BF16, kind="ExternalInput")
    wqT_d = nc.dram_tensor("wqT", [128, KT_C * CH], BF16, kind="ExternalInput")
    wkT_d = nc.dram_tensor("wkT", [128, KT_C * CH], BF16, kind="ExternalInput")
    wvT_d = nc.dram_tensor("wvT", [128, KT_C * CH], BF16, kind="ExternalInput")
    pwT_d = nc.dram_tensor("pwT", [128, KT_C * C], BF16, kind="ExternalInput")
    pbb_d = nc.dram_tensor("pbb", [128, C], BF16, kind="ExternalInput")
    out_d = nc.dram_tensor("out", [B, QB, TOKG, C], F32, kind="ExternalOutput")

    lp = nc.allow_low_precision("bf16 attention pipeline")

    with tile.TileContext(nc) as tc:
        with (tc.tile_pool(name="sb", bufs=1) as sb,
              tc.tile_pool(name="ps", bufs=1, space="PSUM") as ps,
              tc.tile_pool(name="dr", bufs=1, space="DRAM") as dr,
              lp):
            # PSUM (8 banks): sst 2x2 + hav0 + hav1 + acc x2

            # ---- PE warmup during the DMA head (HAM clock gate) ----
            wrm = sb.tile([128, 64], BF16, tag="wrm", bufs=1)
            nc.vector.memset(wrm[:], 0)
            wrm2 = sb.tile([128, 512], BF16, tag="wrm2", bufs=1)
            nc.vector.memset(wrm2[:], 0)
            wacc = ps.tile([128, 64], F32, tag="acc", bufs=2, name="warmacc")
            for i in range(48):
                nc.tensor.matmul(wacc[0:64, :], wrm[:, 0:64], wrm[:, 0:64],
                                 start=True, stop=True)

            # ---- weights (small ones before x; pw queued after x) ----
            wq = sb.tile([128, KT_C, CH], BF16, tag="wq", bufs=1, name="wq")
            wk = sb.tile([128, KT_C, CH], BF16, tag="wk", bufs=1, name="wk")
            wv = sb.tile([128, KT_C, CH], BF16, tag="wv", bufs=1, name="wv")
            for t, d_ in ((wq, wqT_d), (wk, wkT_d), (wv, wvT_d)):
                nc.sync.dma_start(t[:].rearrange("p a b -> p (a b)"), d_.ap())
            pbb_sb = sb.tile([128, C], BF16, tag="pbb", bufs=1)
            nc.sync.dma_start(pbb_sb[:], pbb_d.ap())

            # prime the collective path early: the first collective pays
            # ~11us trigger setup + slow ring warmup (37us observed); a tiny
            # dummy AllToAll during the DMA head absorbs that cost
            prime_in = dr.tile([n_cores, 1, 64], BF16, tag="primein", bufs=1,
                               name="primein")
            prime_out = dr.tile([n_cores, 1, 64], BF16, tag="primeout", bufs=1,
                                name="primeout")
            prime_sb = sb.tile([1, n_cores * 64], BF16, tag="primesb", bufs=1)
            nc.vector.memset(prime_sb[:], 0)
            nc.gpsimd.dma_start(
                prime_in[:].rearrange("a b c -> (a b) c"),
                prime_sb[:].rearrange("a (b c) -> (a b) c", b=n_cores))
            nc.gpsimd.collective_compute(
                "AllToAll", mybir.AluOpType.bypass,
                replica_groups=[list(range(n_cores))],
                ins=[prime_in.opt()], outs=[prime_out.opt()])

            # vau: [key-in-chunk, head, chunk, 65] bf16; col 64 = 1.0
            vau = sb.tile([128, HPC, NK, 65], BF16, tag="vau", bufs=1,
                          name="vau")
            nc.vector.memset(vau[:, :, :, 64:65], 1.0)

            kts = [sb.tile([128, N], BF16, tag=f"kts{hp}", bufs=1,
                           name=f"kts{hp}") for hp in range(2)]
            qt = [sb.tile([128, N], BF16, tag=f"qt{hp}", bufs=1,
                          name=f"qt{hp}") for hp in range(2)]

            pw = []
            pwt = sb.tile([128, KT_C * C], BF16, tag="pw", bufs=1, name="pw")
            for k in range(KT_C):
                pw.append(pwt[:, C * k:C * k + C])


            def qkv_gen():
                # x tiles: [128, 512] per (quarter, chunk): the prewave
                # chains (K(0,0), Q(0,0), V0-3) need only quarter 0, so the
                # first exp fires ~3us after the first 1MB lands instead of
                # waiting for a full half
                xth = [[None] * KT_C for _ in range(4)]
                for hx in range(4):
                    for k in range(KT_C):
                        t = sb.tile([128, 512], BF16, tag="xt", bufs=32,
                                    name=f"xt{hx}_{k}")
                        nc.sync.dma_start(
                            t[:], xT_d.ap()[128 * k:128 * k + 128,
                                            512 * hx:512 * hx + 512])
                        xth[hx][k] = t
                # pw is large and not needed until the first proj: after x
                nc.sync.dma_start(pwt[:], pwT_d.ap())

                def xs(k, col0, w):
                    hx = col0 // 512
                    lo = col0 - 512 * hx
                    return xth[hx][k][:, lo:lo + w]

                def kq_chain(w_t, dst, hp, qb):
                    acc = ps.tile([128, 512], F32, tag="acc", bufs=2,
                                  name=f"kqacc{hp}_{qb}")
                    for k in range(KT_C):
                        nc.tensor.matmul(
                            acc[:], w_t[:, k, 128 * hp:128 * hp + 128],
                            xs(k, 512 * qb, 512),
                            start=(k == 0), stop=(k == KT_C - 1))
                        yield
                    nc.vector.tensor_copy(dst[:, 512 * qb:512 * qb + 512],
                                          acc[:])

                def v_chain(tt):
                    acc = ps.tile([128, CH], F32, tag="acc", bufs=2,
                                  name=f"vacc{tt}")
                    for k in range(KT_C):
                        nc.tensor.matmul(
                            acc[:], xs(k, 128 * tt, 128), wv[:, k, :],
                            start=(k == 0), stop=(k == KT_C - 1))
                        yield
                    nc.vector.tensor_copy(
                        vau[:, :, tt, 0:64],
                        acc[:].rearrange("p (h c) -> p h c", h=HPC))

                # minimal prewave: only what exp chunk 0 needs (K block 0,
                # Q block 0) plus the first PV's V chunks
                yield from kq_chain(wk, kts[0], 0, 0)
                yield from kq_chain(wq, qt[0], 0, 0)
                yield from v_chain(0)
                yield from v_chain(1)
                yield "prewave"
                # fillers, ordered by wave-0 need times (V_c at chunk c,
                # K block c//4 at chunks 4/8/12)
                yield from v_chain(2)
                yield from v_chain(3)
                yield from kq_chain(wk, kts[0], 0, 1)
                yield from v_chain(4)
                yield from v_chain(5)
                yield from v_chain(6)
                yield from kq_chain(wk, kts[0], 0, 2)
                yield from v_chain(7)
                yield from v_chain(8)
                yield from v_chain(9)
                yield from kq_chain(wk, kts[0], 0, 3)
                yield from v_chain(10)
                yield from v_chain(11)
                yield from v_chain(12)
                yield from kq_chain(wq, qt[0], 0, 1)
                yield from v_chain(13)
                yield from v_chain(14)
                yield from v_chain(15)
                yield from kq_chain(wk, kts[1], 1, 0)
                yield from kq_chain(wk, kts[1], 1, 1)
                yield from kq_chain(wk, kts[1], 1, 2)
                yield from kq_chain(wk, kts[1], 1, 3)
                yield from kq_chain(wq, qt[1], 1, 0)
                yield from kq_chain(wq, qt[1], 1, 1)
                yield from kq_chain(wq, qt[0], 0, 2)
                yield from kq_chain(wq, qt[1], 1, 2)
                yield from kq_chain(wq, qt[0], 0, 3)
                yield from kq_chain(wq, qt[1], 1, 3)

            def pull(g, n):
                # returns (got, alive); a "pause" yield stops this round
                # without counting against the budget or killing the gen
                if g is None:
                    return 0, False
                got = 0
                for _ in range(n):
                    v = next(g, "done")
                    if v == "done":
                        return got, False
                    if v == "pause":
                        return got, True
                    got += 1
                return got, True

            def pull_until(g, marker):
                for v in g:
                    if v == marker:
                        return

            fillers = []

            def pull_fillers(n):
                want = n
                ordered = ([e for e in fillers if e[2] == 'proj'] +
                           [e for e in fillers if e[2] != 'proj'])
                for ent in ordered:
                    if ent[0] is None:
                        continue
                    if ent[1] > 0:
                        ent[1] -= 1
                        continue
                    got, alive = pull(ent[0], want)
                    if not alive:
                        ent[0] = None
                    want -= got
                    if want <= 0:
                        break

            def drain_fillers(kind=None):
                for ent in fillers:
                    if kind is not None and ent[2] != kind:
                        continue
                    if ent[1] > 0:
                        ent[1] = 0
                    while ent[0] is not None:
                        got, alive = pull(ent[0], 10 ** 9)
                        if not alive:
                            ent[0] = None

            a2a_in = {}
            wave_no = [0]

            def attention_wave(wi, qb, hp):
                rate = 4 if wi < 4 else 2
                hav = [ps.tile([65, 512], F32, tag=f"hav{h}", bufs=1,
                               name=f"hav{qb}_{hp}_{h}")
                       for h in range(2)]
                for c in range(NK):
                    sst = ps.tile([128, 1024], F32, tag="sst", bufs=2,
                                  name=f"sst{qb}_{hp}_{c}")
                    pt = sb.tile([128, 2, 512], BF16, tag="pt", bufs=8,
                                 name=f"pt{qb}_{hp}_{c}")
                    for h2 in range(2):
                        hs = slice(64 * h2, 64 * h2 + 64)
                        nc.tensor.matmul(
                            sst[:, 512 * h2:512 * h2 + 512],
                            kts[hp][hs, 128 * c:128 * c + 128],
                            qt[hp][hs, 512 * qb:512 * qb + 512],
                            start=True, stop=True)
                    nc.scalar.activation(pt[:].rearrange("p a b -> p (a b)"),
                                         sst[:], EXP, scale=scale)
                    for h2 in range(2):
                        nc.tensor.matmul(
                            hav[h2][:],
                            vau[:, 2 * hp + h2, c, 0:65],
                            pt[:, h2, :],
                            start=(c == 0), stop=(c == NK - 1))
                    pull_fillers(rate)
                # copy h_aug out of PSUM fast so hav slots recycle
                hcp = [sb.tile([65, 512], F32, tag=f"hcp{h}", bufs=3,
                               name=f"hcp{qb}_{hp}_{h}")
                       for h in range(2)]
                for h2 in range(2):
                    nc.vector.tensor_copy(hcp[h2][:], hav[h2][:])
                # normalize: reciprocal of [1,512] rows via DMA-reshape to
                # [128,4] (multi-partition DVE), then broadcast and mul
                ht = sb.tile([128, 512], BF16, tag="ht", bufs=2,
                             name=f"ht{qb}_{hp}")
                dnt = sb.tile([128, 8], F32, tag="dnt", bufs=2,
                              name=f"dnt{qb}_{hp}")
                rnt = sb.tile([128, 8], F32, tag="rnt", bufs=2,
                              name=f"rnt{qb}_{hp}")
                for h2 in range(2):
                    nc.sync.dma_start(dnt[:, 4 * h2:4 * h2 + 4],
                                      hcp[h2][64:65, :])
                nc.vector.reciprocal(rnt[:], dnt[:])
                for h2 in range(2):
                    nrr = sb.tile([1, 512], F32, tag=f"nrr{h2}", bufs=2,
                                  name=f"nrr{qb}_{hp}_{h2}")
                    nc.sync.dma_start(nrr[:], rnt[:, 4 * h2:4 * h2 + 4])
                    bcs = sb.tile([64, 512], F32, tag=f"bcs{h2}", bufs=2,
                                  name=f"bcs{qb}_{hp}_{h2}")
                    nc.gpsimd.partition_broadcast(bcs[:], nrr[:])
                    nc.vector.tensor_mul(ht[64 * h2:64 * h2 + 64, :],
                                         hcp[h2][0:64, :], bcs[:])
                # scatter into this (group, hp)'s A2A input buffer
                a2a_in[(qb, hp)] = dr.tile([n_cores, 128, TOKG], BF16,
                                           tag=f"a2ain{qb}_{hp}", bufs=2,
                                           name=f"a2ain{qb}_{hp}")
                dst = a2a_in[(qb, hp)].transpose([1, 0, 2])
                nc.sync.dma_start(
                    dst, ht[:].rearrange("c (j t) -> c j t", j=n_cores))

            def proj_gen(g, a2a_outs, mm_wave):
                # one batched load per parity: plall[:, k, b, t] with
                # k = 2*k2 + par from a2a_outs[par][4b + k2, :, :]
                plall = sb.tile([128, KT_C, 2, TOKG], BF16, tag="plall",
                                bufs=2, name=f"plall{g}")
                for par in range(2):
                    for b in range(B):
                        dst = plall[:, par::2, b, :]
                        srcp = a2a_outs[par][4 * b:4 * b + 4, 0:128, :]
                        nc.scalar.dma_start(dst, srcp.transpose([1, 0, 2]))
                pl = [plall[:, k, :, :] for k in range(KT_C)]
                # the proj MMs enter the in-order PE queue; if emitted while
                # the collective is still in flight their pl wait stalls
                # every matmul behind them -> gate emission on the wave clock
                while wave_no[0] < mm_wave:
                    yield "pause"
                for _ in range(6):
                    yield "pause"
                for oh in range(2):
                    os_ = slice(512 * oh, 512 * oh + 512)
                    acc = ps.tile([128, 512], F32, tag="acc", bufs=2,
                                  name=f"pacc{g}_{oh}")
                    for k in range(KT_C):
                        nc.tensor.matmul(acc[:], pl[k][:], pw[k][:, os_],
                                         start=(k == 0), stop=(k == KT_C - 1))
                        yield
                    osb = sb.tile([128, 512], F32, tag="osb", bufs=2,
                                  name=f"osb{g}_{oh}")
                    nc.vector.tensor_add(osb[:], acc[:], pbb_sb[:, os_])
                    for b in range(B):
                        nc.scalar.dma_start(
                            out_d.ap()[b, g, :, os_],
                            osb[TOKG * b:TOKG * b + TOKG, :])
                    yield

            gen = qkv_gen()
            pull_until(gen, "prewave")
            fillers.append([gen, 0, 'qkv'])

            a2a_out = {}
            hp_done = {}
            for wi, (qb, hp) in enumerate(WAVES):
                attention_wave(wi, qb, hp)
                # half-A2A for this (qb, hp) right away: halves the
                # tail-critical collective and overlaps the hp0 half a
                # wave earlier
                a2a_out[(qb, hp)] = dr.tile([n_cores, 128, TOKG], BF16,
                                            tag=f"a2aout{qb}_{hp}", bufs=2,
                                            name=f"a2aout{qb}_{hp}")
                nc.gpsimd.collective_compute(
                    "AllToAll", mybir.AluOpType.bypass,
                    replica_groups=[list(range(n_cores))],
                    ins=[a2a_in[(qb, hp)].opt()],
                    outs=[a2a_out[(qb, hp)].opt()])
                hp_done.setdefault(qb, set()).add(hp)
                if len(hp_done[qb]) == 2:
                    fillers.append([proj_gen(
                        qb, (a2a_out[(qb, 0)], a2a_out[(qb, 1)]), wi + 2),
                        2, 'proj'])
                wave_no[0] = wi + 1
                pull_fillers(4)
            wave_no[0] = 99
            # keep the PE warm while the last A2A lands (an idle window
            # >3.4us re-throttles HAM to 1.2GHz, slowing the final proj)
            for i in range(140):
                nc.tensor.matmul(wacc[0:64, :], wrm[:, 0:64], wrm[:, 0:64],
                                 start=True, stop=True)
            drain_fillers()

    nc.compile()
    return nc


def shard_inputs(x, qkv_w, proj_w, proj_b, n_cores=N_CORES):
    """Host-side sharding for (batch, head-quad) layout."""
    bf = ml_dtypes.bfloat16
    x = np.asarray(x)
    qkv_w = np.asarray(qkv_w)

    def pack(wT):
        cdim, cols = wT.shape
        return np.ascontiguousarray(
            wT.reshape(cdim // 128, 128, cols).transpose(1, 0, 2)
            .reshape(128, -1)).astype(bf)

    pwT = pack(np.asarray(proj_w).T)
    pbb = np.ascontiguousarray(
        np.tile(np.asarray(proj_b)[None, :], (128, 1))).astype(bf)
    xTb = [np.ascontiguousarray(np.asarray(x[b]).T).astype(bf)
           for b in range(B)]
    in_maps = []
    for i in range(n_cores):
        b, hq = i // 4, i % 4
        cs = slice(CH * hq, CH * hq + CH)
        in_maps.append({
            "xT": xTb[b],
            "wqT": pack(qkv_w[cs, :].T),
            "wkT": pack(qkv_w[C:][cs, :].T),
            "wvT": pack(qkv_w[2 * C:][cs, :].T),
            "pwT": pwT,
            "pbb": pbb,
        })
    return in_maps


def assemble_output(res, N, n_cores=N_CORES):
    QB = N // 512
    TOKG = 512 // n_cores
    out = np.empty((B, N, C), dtype=np.float32)
    for j in range(n_cores):
        o = res.results[j]["out"]  # [B, QB, TOKG, C]
        for b in range(B):
            for g in range(QB):
                lo = 512 * g + TOKG * j
                out[b, lo:lo + TOKG, :] = o[b, g]
    return out


_NC_CACHE = {}


def _get_program(N):
    if N not in _NC_CACHE:
        _NC_CACHE[N] = build_program(N=N)
    return _NC_CACHE[N]


_WARMED = set()


def kernel(x, qkv_w, proj_w, proj_b):
    x = np.asarray(x)
    Bx, N, Cx = x.shape
    assert (Bx, Cx) == (B, C), (Bx, Cx)
    nc = _get_program(N)
    in_maps = shard_inputs(x, qkv_w, proj_w, proj_b)
    if N not in _WARMED:
        # first device execution after NEFF load has a cold-start race in
        # the collective path (observed: first run occasionally returns
        # garbage in the earliest AllToAll groups); warm up and discard
        run_bass_kernel_spmd(nc, in_maps, list(range(N_CORES)))
        _WARMED.add(N)
    res = run_bass_kernel_spmd(nc, in_maps, list(range(N_CORES)))
    return assemble_output(res, N)
